# revision 30
# baseline (speedup 1.0000x reference)
"""Bass/Trainium2 kernel for ragged masked attention (8-core data parallel).

reference math:
    e[b,t] = (W @ enc[b,t] + bias) . query[b]   for t <= tgt_index[b]
    ctx[b] = softmax_t(e[b, :L_b]) @ enc[b, :L_b],  L_b = tgt_index[b]+1

Key identities / design (shaped by HW microbenchmarks):
  * e[b,t] = enc[b,t,:] . (W^T query[b]) + const_b ; softmax is shift
    invariant so the bias term drops out entirely.
  * enc streams as fp16 (halves HBM traffic; ~2e-3 relative l2 error).
  * DVE fast paths only engage on flat 2D access patterns, so every
    vector op here uses 2D slices of 2D tiles. qW is materialized twice
    per slot ([P, 2H]) so the multiply runs 2 tiles wide with no
    broadcast AP (301 ns/tile measured vs 925 with 3D broadcast).
  * Row-sum of the products splits between two lanes balanced by
    measured cadence: DVE tensor_scalar+accum_out (607 ns) and ACT
    activation Identity+accum_out (957 ns).
  * exp is shifted by the exact slot max (device transpose+reduce);
    required because fp16 x would underflow under a loose host bound.
  * x stays unnormalized through the PE context matmuls; 1/total is
    folded into the per-slot PSUM->SBUF copy (scalar.mul / DVE
    tensor_scalar_mul), halving the tail work.
  * Rows past L_b are filled with -1e4 * qW/||qW||^2 so their energy is
    -1e4 -> exp == 0: self-masking, no mask tensor needed.

Per-core schedule: batches sorted by tile count, slot s on every core has
the same (max) tile count NT[s] -> all 8 cores run one identical graph.
"""
import numpy as np

B, T, H, Q = 64, 2048, 512, 512
P = 128                       # SBUF partitions / t-tile height
NCORES = 8
NSLOTS = B // NCORES          # 8 batch slots per core
CHUNK = 8                     # t-tiles per DMA (8*128KB = 1MiB transfers)


# ---------------------------------------------------------------- BIR patch
def _split_waits(bir: dict) -> dict:
    """This walrus build accepts only one sem wait/update per CTRL
    instruction; split Tile's multi-wait drains into single-wait chains."""
    uid = [0]

    def fresh(name):
        uid[0] += 1
        return f"{name}_sw{uid[0]}"

    for fn in bir.get("functions", []):
        for blk in fn.get("blocks", []):
            out = []
            for inst in blk.get("instructions", []):
                si = inst.get("sync_info")
                if si:
                    ow = si.get("on_wait") or []
                    if len(ow) > 1:
                        for w in ow[:-1]:
                            out.append({
                                "debug": inst.get("debug", 0),
                                "engine": inst["engine"],
                                "ins": [], "outs": [],
                                "name": fresh(inst["name"]),
                                "opcode": "EventSemaphore",
                                "sync_info": {"on_update": [], "on_wait": [w]},
                            })
                        si["on_wait"] = [ow[-1]]
                out.append(inst)
                if si:
                    ou = si.get("on_update") or []
                    if len(ou) > 1:
                        si["on_update"] = [ou[0]]
                        for u in ou[1:]:
                            out.append({
                                "debug": inst.get("debug", 0),
                                "engine": inst["engine"],
                                "ins": [], "outs": [],
                                "name": fresh(inst["name"]),
                                "opcode": "EventSemaphore",
                                "sync_info": {"on_update": [u], "on_wait": []},
                            })
            blk["instructions"] = out
    return bir


_patched = False


def _install_bir_patch():
    global _patched
    if _patched:
        return
    import json
    from concourse import bass2jax, bass_utils
    orig = bass_utils.compile_bir_kernel

    def patched(bir_json, tmpdir, neff_name="file.neff"):
        bir = json.loads(bir_json)
        bir = _split_waits(bir)
        return orig(json.dumps(bir).encode(), tmpdir, neff_name=neff_name)

    bass2jax.compile_bir_kernel = patched
    _patched = True


# ---------------------------------------------------------------- builder
SKIP_TAIL_BARRIER = True   # replace Tile's ~16us tail barrier w/ bare drain
DVE_RED = (1, 3, 5)        # tile%7 slots whose reduce runs on DVE (3 of 7)
# process the smallest slot first so the PE/exp pipeline fills quickly,
# then largest-to-smallest (enc is host-packed in this same order)
SLOT_ORDER = (7, 0, 1, 2, 3, 4, 5, 6)


def _minimal_drain_and_barrier(self, tick_clock, wait_clock):
    """Tail: one drain on Sync waiting on the global clock (covers the
    final output DMA); skip the two all-engine EVSEM barriers and the
    semaphore clears (~16us on silicon, pointless for a one-shot NEFF)."""
    from concourse.vector_clock import ScopedClock
    drain_inst = self.nc.sync.drain()
    wait_clock.add_sem_waits(
        drain_inst.ins, ScopedClock({None: tick_clock.global_clock})
    )
    popped = self.nc._tile_sem_poison_stack.pop()
    assert popped is self._sem_poison


def build_graph(NT, chunk=CHUNK):
    """One SPMD graph; NT[s] = tile count of slot s (same on all cores)."""
    from concourse import bass, tile, mybir

    if SKIP_TAIL_BARRIER:
        tile.TileContext._drain_and_barrier = _minimal_drain_and_barrier

    TOT = sum(NT)
    f32 = mybir.dt.float32
    f16 = mybir.dt.float16
    nc = bass.Bass()
    # encp is partition-major fp16: [128, TOT*512]; slot s = cols off_s*512..
    # -> every DMA is 128 long contiguous runs (one per partition)
    QC = Q // P  # 4 contraction chunks for query@W

    # all small tensors arrive pre-permuted from host so every DMA is a
    # contiguous [P, N] block copy (no sub-KB scatter packets)
    encp = nc.declare_dram_parameter("encp", [P, TOT * H], f16, isOutput=False)
    qt = nc.declare_dram_parameter("queryT", [P, QC * NSLOTS], f16,
                                   isOutput=False)
    qtrep = nc.declare_dram_parameter("qtrep", [P, QC * 2 * P], f16,
                                      isOutput=False)
    w = nc.declare_dram_parameter("w", [P, QC * H], f16, isOutput=False)
    ident = nc.declare_dram_parameter("ident", [P, P], f32, isOutput=False)
    outp = nc.declare_dram_parameter("out", [1, NSLOTS * H], f32, isOutput=True)

    with tile.TileContext(nc) as tc:
        with (
            tc.tile_pool(name="const", bufs=1) as constp,
            tc.tile_pool(name="wpool", bufs=1) as wpool,
            tc.tile_pool(name="enc", bufs=10) as encpool,
            tc.tile_pool(name="small", bufs=3) as small,
            tc.tile_pool(name="prod", bufs=4) as prodp,
            tc.tile_pool(name="ps", bufs=2, space="PSUM") as psp,
            tc.tile_pool(name="psmisc", bufs=2, space="PSUM") as psmisc,
            tc.tile_pool(name="psone", bufs=1, space="PSUM") as psone,
        ):
            ones_col = constp.tile([P, 1], f32)       # lhsT for total
            nc.vector.memset(ones_col[:], 1.0)
            ones_row = constp.tile([1, P], f32)       # lhsT for max bcast
            nc.vector.memset(ones_row[:], 1.0)
            ident_sb = constp.tile([P, P], f32)

            # small inputs first on the scalar queue (contiguous copies);
            # W/qtrep lead since the qW matmul chain gates the first
            # multiply; ident is not needed until the first slot max
            w_sb = wpool.tile([P, QC, H], f16)
            qt_sb = wpool.tile([P, QC, NSLOTS], f16)
            qtrep_sb = wpool.tile([P, QC, 2, P], f16)
            for c in range(QC):      # W in 4 chunks so qW matmuls pipeline
                nc.scalar.dma_start(w_sb[:, c, :], w[:, c * H:(c + 1) * H])
            nc.scalar.dma_start(qtrep_sb[:], qtrep[:].rearrange(
                "p (c s k) -> p c s k", c=QC, s=2))
            nc.scalar.dma_start(qt_sb[:], qt[:].rearrange(
                "p (c b) -> p c b", c=QC))
            nc.scalar.dma_start(ident_sb[:], ident[:])

            # enc chunk tiles (2D [P, chunk*H]); the first processed slot's
            # first chunk is split in half so its consumers start sooner
            enc_tiles = {}

            def issue_chunk(s, k, off_s):
                ctiles = min(chunk, NT[s] - k * chunk)
                et = encpool.tile([P, chunk * H], f16, tag="enc")
                nc.sync.dma_start(
                    et[:, :ctiles * H],
                    encp[:, (off_s + k * chunk) * H:
                         (off_s + k * chunk + ctiles) * H])
                enc_tiles[(s, k)] = (et, ctiles)

            issue_chunk(SLOT_ORDER[0], 0, 0)

            # first two processed slots: qW broadcast computed straight into
            # PSUM by the PE (host-replicated query columns), copied twice
            # to SBUF fp16 so the DVE multiplies run 2 tiles wide on flat
            # 2D APs - not gated on the dram bounce below
            qwb01 = []
            for pos in range(2):
                qps = psmisc.tile([P, H], f32, tag="qwb01")
                for c in range(QC):
                    nc.tensor.matmul(qps[:], qtrep_sb[:, c, pos, :],
                                     w_sb[:, c, :],
                                     start=(c == 0), stop=(c == QC - 1))
                qsb = wpool.tile([P, 2 * H], f16)
                nc.scalar.copy(qsb[:, :H], qps[:])
                nc.scalar.copy(qsb[:, H:], qps[:])
                qwb01.append(qsb)

            # qW[s, h] for all 8 slots in one accumulated matmul chain, then
            # each row replicated to [P, 2H] (2 copies) via broadcast DMA
            qw_ps = psone.tile([NSLOTS, H], f32, tag="qwps")
            for c in range(QC):
                nc.tensor.matmul(qw_ps[:], qt_sb[:, c, :], w_sb[:, c, :],
                                 start=(c == 0), stop=(c == QC - 1))
            qw_sb = wpool.tile([NSLOTS, H], f16)
            nc.scalar.copy(qw_sb[:], qw_ps[:])
            qw_dram = nc.dram_tensor("qw_dram", [NSLOTS, H], f16)
            nc.scalar.dma_start(qw_dram[:], qw_sb[:])
            # broadcasts ride the idle GpSimd queue so their sem waits never
            # stall the enc-chunk (sync) or reduce (scalar) queues
            qwb_all = wpool.tile([P, (NSLOTS - 2) * 2 * H], f16)
            for pos in range(2, NSLOTS):
                s = SLOT_ORDER[pos]
                nc.gpsimd.dma_start(
                    qwb_all[:, (pos - 2) * 2 * H:(pos - 1) * 2 * H]
                    .rearrange("p (n d) -> p n d", d=H),
                    qw_dram[s:s + 1, :][None].to_broadcast((P, 2, H)))

            out_sb = wpool.tile([1, NSLOTS * H], f32)
            tcount = [0]   # global tile counter for reduce-lane routing

            off = 0
            for pos in range(NSLOTS):
                s = SLOT_ORDER[pos]
                nt = NT[s]
                qwb2 = (qwb01[pos][:] if pos < 2 else
                        qwb_all[:, (pos - 2) * 2 * H:(pos - 1) * 2 * H])

                # ragged-packed encoder cols for this slot, chunked DMAs
                nchunks = (nt + chunk - 1) // chunk
                for k in range(nchunks):
                    if (s, k) not in enc_tiles:
                        issue_chunk(s, k, off)
                chunks = [enc_tiles[(s, k)] for k in range(nchunks)]
                off += nt

                # energies: e[:, j] = sum_h enc_tile_j * qW
                # DVE multiply 2 tiles/op (flat 2D), then per tile either
                # DVE tensor_scalar+accum (reduce) or ACT Identity+accum
                e_buf = small.tile([P, nt], f32, tag="ebuf")
                for k, (et, ctiles) in enumerate(chunks):
                    j = 0
                    while j < ctiles:
                        ji = k * chunk + j
                        g = min(2, ctiles - j)
                        prod = prodp.tile([P, 2 * H], f16, tag="prod")
                        nc.vector.tensor_mul(prod[:, :g * H],
                                             et[:, j * H:(j + g) * H],
                                             qwb2[:, :g * H])
                        for jj in range(g):
                            pj = prod[:, jj * H:(jj + 1) * H]
                            colj = e_buf[:, ji + jj:ji + jj + 1]
                            if tcount[0] % 7 in DVE_RED:
                                nc.vector.tensor_reduce(
                                    colj, pj, axis=mybir.AxisListType.X,
                                    op=mybir.AluOpType.add)
                            else:
                                ascr = prodp.tile([P, H], f16, tag="ascr")
                                nc.scalar.activation(
                                    ascr[:], pj,
                                    mybir.ActivationFunctionType.Identity,
                                    bias=0.0, scale=1.0, accum_out=colj)
                            tcount[0] += 1
                        j += g

                # exact slot max of e via transpose + cross-partition reduce,
                # broadcast (negated) to all partitions as the exp bias
                rmax = small.tile([P, 1], f32, tag="rmax")
                nc.vector.reduce_max(rmax[:], e_buf[:],
                                     axis=mybir.AxisListType.X)
                rmT = psone.tile([1, P], f32, tag="mx")
                nc.tensor.transpose(rmT[:], rmax[:], ident_sb[:])
                gneg = small.tile([1, 1], f32, tag="gneg")
                nc.vector.tensor_reduce(gneg[:], rmT[:],
                                        axis=mybir.AxisListType.X,
                                        op=mybir.AluOpType.max,
                                        negate=True)
                bb_ps = psone.tile([P, 1], f32, tag="mx")
                nc.tensor.matmul(bb_ps[:], ones_row[:], gneg[:],
                                 start=True, stop=True)
                bias_sb = small.tile([P, 1], f32, tag="bias")
                nc.vector.tensor_copy(bias_sb[:], bb_ps[:])

                # x = exp(e - max) in fp16; per-partition row sums in f32
                x_s = small.tile([P, nt], f16, tag="xs")
                srow = small.tile([P, 1], f32, tag="srow")
                nc.scalar.activation(x_s[:], e_buf[:],
                                     mybir.ActivationFunctionType.Exp,
                                     bias=bias_sb[:], scale=1.0,
                                     accum_out=srow[:])

                # total = sum over partitions; rinv = 1/total (runs on the
                # side while the PE accumulates the context)
                tot_ps = psmisc.tile([1, 1], f32, tag="tot")
                nc.tensor.matmul(tot_ps[:], ones_col[:], srow[:],
                                 start=True, stop=True)
                rinv = small.tile([1, 1], f32, tag="rinv")
                nc.vector.reciprocal(rinv[:], tot_ps[:])

                # context: ctx[h] = sum_t x[t] enc[t, h], fp16 matmuls
                # accumulated in PSUM
                ctx_ps = psp.tile([1, H], f32)
                ji = 0
                for k, (et, ctiles) in enumerate(chunks):
                    for j in range(ctiles):
                        nc.tensor.matmul(ctx_ps[:],
                                         x_s[:, ji:ji + 1],
                                         et[:, j * H:(j + 1) * H],
                                         start=(ji == 0), stop=(ji == nt - 1))
                        ji += 1

                # out[s] = ctx * rinv, folded into the PSUM->SBUF copy;
                # alternate ACT/DVE to balance the tails
                oslice = out_sb[:, s * H:(s + 1) * H]
                if s % 2 == 0:
                    nc.scalar.mul(oslice, ctx_ps[:], rinv[:])
                else:
                    nc.vector.tensor_scalar_mul(oslice, ctx_ps[:], rinv[:])
                nc.gpsimd.dma_start(outp[:, s * H:(s + 1) * H], oslice)

    return nc


# ---------------------------------------------------------------- host side
TRACE = False       # test.py sets True to capture a profile
LAST_RES = None     # BassKernelResults of the last run (exec_time_ns etc.)


def kernel(query, encoder_outputs, W, b, tgt_index):
    global LAST_RES
    _install_bir_patch()
    from concourse.bass_utils import run_bass_kernel_spmd

    f16 = np.float16
    query = np.asarray(query, dtype=np.float32)
    enc = np.ascontiguousarray(np.asarray(encoder_outputs, dtype=np.float32))
    W_ = np.asarray(W, dtype=np.float32)
    tgt = np.asarray(tgt_index).astype(np.int64)

    L = np.clip(tgt + 1, 1, T).astype(np.int64)          # valid lengths
    nt = ((L + P - 1) // P).astype(np.int64)             # tiles per batch

    # slot grouping: sort batches by tile count (desc); slot s gets ranks
    # [s*8, s*8+8); every core's slot s then has NT[s] = max tiles in group
    order = np.argsort(-nt, kind="stable")
    NT = [int(nt[order[s * NCORES]]) for s in range(NSLOTS)]
    TOT = sum(NT)

    # host-side qW only for the self-masking pad rows; device recomputes
    # qW itself for the actual math.
    # reference proj = einsum('bth,qh->btq', enc, W) -> W[q, h];
    # energies = sum_q proj[b,t,q] query[b,q] = enc . (query @ W)
    qW = query @ W_                                       # [B, H]
    qnorm = np.linalg.norm(qW, axis=1)                    # [B]
    # pad row vector per batch: dot with qW == -1e4
    safe = np.maximum(qnorm, 1e-30) ** 2
    padrow = ((-1.0e4 / safe)[:, None] * qW).astype(f16)  # [B, H] fp16

    # W pre-permuted to the device layout [P, QC, H] (chunk-major)
    QC = Q // P
    w_dev = np.ascontiguousarray(
        W_.astype(f16).reshape(QC, P, H).transpose(1, 0, 2)).reshape(P, QC * H)

    in_maps = []
    placement = np.empty((NCORES, NSLOTS), dtype=np.int64)
    for i in range(NCORES):
        # partition-major packing: encp[p, (off+j)*H + h] = row j*128+p of
        # the slot's padded prefix -> each DMA reads 128 contiguous runs
        encp = np.empty((P, TOT * H), dtype=f16)
        qt = np.empty((P, QC, NSLOTS), dtype=f16)
        off = 0
        for s in SLOT_ORDER:
            bidx = int(order[s * NCORES + i])
            placement[i, s] = bidx
            lb, ntb = int(L[bidx]), NT[s]
            block = np.empty((ntb * P, H), dtype=f16)
            block[:lb] = enc[bidx, :lb].astype(f16)
            block[lb:] = padrow[bidx]
            encp[:, off * H:(off + ntb) * H] = (
                block.reshape(ntb, P, H).transpose(1, 0, 2).reshape(P, ntb * H))
            qt[:, :, s] = query[bidx].astype(f16).reshape(QC, P).T
            off += ntb
        qtrep = np.empty((P, QC, 2, P), dtype=f16)
        qtrep[:, :, 0, :] = qt[:, :, SLOT_ORDER[0]:SLOT_ORDER[0] + 1]
        qtrep[:, :, 1, :] = qt[:, :, SLOT_ORDER[1]:SLOT_ORDER[1] + 1]
        im = {
            "encp": encp,
            "queryT": qt.reshape(P, QC * NSLOTS),
            "qtrep": qtrep.reshape(P, QC * 2 * P),
            "w": w_dev,
            "ident": np.eye(P, dtype=np.float32),
        }
        in_maps.append(im)

    nc = build_graph(tuple(NT))
    res = run_bass_kernel_spmd(nc, in_maps, core_ids=list(range(NCORES)),
                               trace=TRACE)
    LAST_RES = res

    out = np.empty((B, H), dtype=np.float32)
    for i in range(NCORES):
        o = np.asarray(res.results[i]["out"]).reshape(NSLOTS, H)
        for s in range(NSLOTS):
            out[placement[i, s]] = o[s]
    return out
